# revision 15
# baseline (speedup 1.0000x reference)
"""MoE routing kernel for Trainium2 (8 NeuronCores, data-parallel over rows).

Problem: x_enc [32,512,128] -> rows x [N=4096, L=512] (N = B*V),
gating top-2-of-16 softmax, dense per-expert Linear(512,720), weighted
combine, output [32, 720, 128].

Strategy (per core, 4 batches = 512 rows):
  * Everything is computed in the "transposed" orientation so no data
    transposes are needed anywhere:
      - x_enc[b] is [L, V]  == x^T columns for batch b    (matmul rhs)
      - expert_w[e] is [L, P] == lhsT for the expert matmul (stationary)
      - output psum [P-tile, rows] == result[b] layout    ([P, V])
  * Gating runs in row-orientation (rows on partitions) where top-2 +
    softmax are native per-partition ops; the resulting [rows, E] gate
    weight matrix is PE-transposed to wT [E, rows].
  * Per-expert gate row-scaling is applied to x^T columns:
    a K=1 ones-matmul broadcasts wT[e] across 128 partitions, DVE
    multiplies x^T tiles by it, and the 16 scaled matmuls accumulate in
    PSUM over experts AND k-tiles, so the top-2 combine is free.
  * Bias: one K=16 matmul  psum += expert_b[:, ptile].T @ wT  per p-tile.
  * Matmuls use float32r (full PE rate at free-dim 512 vs 4x slower fp32).
"""

import os
import numpy as np

import concourse.bass as bass
import concourse.tile as tile
from concourse import bacc, mybir
from concourse.bass_utils import run_bass_kernel_spmd
from concourse.masks import make_identity

B, L, V = 32, 512, 128
E, P = 16, 720
N_CORES = 8
BPC = B // N_CORES          # batches per core
NROW = BPC * V              # rows per core = 512 (matmul free dim)
KT = L // 128               # 4 k-tiles
PT = (P + 127) // 128       # 6 p-tiles (5x128 + 80)
PW = [min(128, P - 128 * p) for p in range(PT)]

F32 = mybir.dt.float32
F32R = mybir.dt.float32r


def _build(ctx, tc, x, gw_d, gb_d, ew_d, eb_d, out_d):
    nc = tc.nc
    AX = mybir.AxisListType.X
    OP = mybir.AluOpType
    AF = mybir.ActivationFunctionType

    const = ctx.enter_context(tc.tile_pool(name="const", bufs=1))
    xt_pool = ctx.enter_context(tc.tile_pool(name="xtp", bufs=KT))
    gate_pool = ctx.enter_context(tc.tile_pool(name="gatep", bufs=2))
    wb_pool = ctx.enter_context(tc.tile_pool(name="wbp", bufs=E))
    ew_pool = ctx.enter_context(tc.tile_pool(name="ewp", bufs=12))
    xs_pool = ctx.enter_context(tc.tile_pool(name="xsp", bufs=8))
    ob_pool = ctx.enter_context(tc.tile_pool(name="obp", bufs=PT))
    acc_psum = ctx.enter_context(tc.tile_pool(name="accp", bufs=PT, space="PSUM"))
    mp_psum = ctx.enter_context(tc.tile_pool(name="mpp", bufs=2, space="PSUM"))

    # ---- constants ----
    ident = const.tile([128, 128], F32, tag="ident")
    make_identity(nc, ident[:])
    ones = const.tile([1, 128], F32, tag="ones")
    nc.gpsimd.memset(ones[:], 1.0)

    # Single-DMA loads (multiple DMA writers per tile blow the per-instruction
    # sync-wait budget of downstream matmuls in walrus codegen).
    gwt = const.tile([128, KT * E], F32, tag="gwt")     # gate_w k-tiles side by side
    nc.sync.dma_start(gwt[:].rearrange("p (l e) -> p l e", l=KT),
                      gw_d.rearrange("(l p) e -> p l e", p=128))
    gbt = const.tile([1, E], F32, tag="gbt")
    nc.sync.dma_start(gbt[:], gb_d[:, :])
    # fp32r-matmul operands must live in float32r locations end-to-end
    # (BIR verifier); DMA is a byte copy so bitcast the DRAM side.
    ebt = const.tile([E, P], F32R, tag="ebt")
    nc.sync.dma_start(ebt[:], eb_d[:, :].bitcast(F32R))

    # ---- x^T tiles: xt[l][p, b*V+v] = x[b, l*128+p, v] ----
    xt = []
    x_lbv = x.rearrange("b (l p) v -> l p b v", p=128)   # [KT, 128, BPC, V]
    for l in range(KT):
        t = xt_pool.tile([128, NROW], F32, tag="xt", name=f"xt{l}")
        nc.sync.dma_start(t[:].rearrange("p (b v) -> p b v", b=BPC), x_lbv[l])
        xt.append(t)

    # ---- gating: gate = x @ gate_w + gate_b, top-2 softmax weights ----
    # wT is consumed by fp32r matmuls, so its compute producer must write
    # fp32r-rounded values (BIR verifier rule); allocate it as float32r.
    wT = const.tile([E, NROW], F32R, tag="wT")          # wT[e, n] = gate weight
    wT_flat = const.tile([1, E * NROW], F32R, tag="wTf")  # same, all on partition 0
    for r in range(NROW // 128):                        # row-tile == batch r
        gps = mp_psum.tile([128, E], F32, tag="mp", name=f"gps{r}")
        for l in range(KT):
            nc.tensor.matmul(gps[:], xt[l][:, r * 128:(r + 1) * 128],
                             gwt[:, l * E:(l + 1) * E], start=(l == 0), stop=False)
        nc.tensor.matmul(gps[:], ones[:], gbt[:], start=False, stop=True)

        g = gate_pool.tile([128, E], F32, tag="g")
        nc.scalar.copy(g[:], gps[:])
        m1 = gate_pool.tile([128, 1], F32, tag="m1")
        nc.vector.reduce_max(m1[:], g[:], axis=AX)
        tb = gate_pool.tile([128, E], F32, tag="tb")
        nc.vector.tensor_scalar(tb[:], g[:], m1[:], 1e30, OP.is_equal, OP.mult)
        g2 = gate_pool.tile([128, E], F32, tag="g2")
        nc.vector.tensor_sub(g2[:], g[:], tb[:])
        m2 = gate_pool.tile([128, 1], F32, tag="m2")
        nc.vector.reduce_max(m2[:], g2[:], axis=AX)
        d = gate_pool.tile([128, 1], F32, tag="d")
        nc.vector.tensor_sub(d[:], m2[:], m1[:])
        s2 = gate_pool.tile([128, 1], F32, tag="s2")
        nc.scalar.activation(s2[:], d[:], AF.Sigmoid)
        s1 = gate_pool.tile([128, 1], F32, tag="s1")
        nc.scalar.activation(s1[:], d[:], AF.Sigmoid, scale=-1.0)
        w1 = gate_pool.tile([128, E], F32, tag="w1")
        nc.vector.tensor_scalar(w1[:], g[:], m1[:], s1[:], OP.is_equal, OP.mult)
        w2 = gate_pool.tile([128, E], F32, tag="w2")
        nc.vector.tensor_scalar(w2[:], g2[:], m2[:], s2[:], OP.is_equal, OP.mult)
        wr = gate_pool.tile([128, E], F32, tag="wr")
        nc.vector.tensor_add(wr[:], w1[:], w2[:])

        tps = mp_psum.tile([E, 128], F32, tag="mp", name=f"tps{r}")
        nc.tensor.transpose(tps[:], wr[:], ident[:])
        nc.vector.tensor_copy(wT[:, r * 128:(r + 1) * 128], tps[:])

    # matmul rhs must start at partition 0/32/64, so flatten wT rows onto
    # partition 0 first (DMA partition->free move), then broadcast from there.
    for e in range(E):
        nc.sync.dma_start(wT_flat[0:1, e * NROW:(e + 1) * NROW], wT[e:e + 1, :])

    # ---- broadcast gate columns across partitions: wb[e] = ones^T @ wT[e] ----
    wb = []
    for e in range(E):
        wps = mp_psum.tile([128, NROW], F32, tag="mp", name=f"wps{e}")
        nc.tensor.matmul(wps[:],
                         ones[:].bitcast(F32R),
                         wT_flat[0:1, e * NROW:(e + 1) * NROW],
                         start=True, stop=True)
        wbt = wb_pool.tile([128, NROW], F32, tag="wb", name=f"wb{e}")
        nc.scalar.copy(wbt[:], wps[:])
        wb.append(wbt)

    # ---- main: psum[p] = sum_e sum_l W_e[l,p].T @ (x^T * wb[e]) ----
    accs = [acc_psum.tile([128, NROW], F32, tag="acc", name=f"acc{p}")
            for p in range(PT)]
    for e in range(E):
        ewt = []
        for l in range(KT):
            w = ew_pool.tile([128, P], F32R, tag="ew", name=f"ew{e}_{l}")
            nc.sync.dma_start(w[:], ew_d[e, l * 128:(l + 1) * 128, :].bitcast(F32R))
            ewt.append(w)
        xst = []
        for l in range(KT):
            s = xs_pool.tile([128, NROW], F32R, tag="xs", name=f"xs{e}_{l}")
            nc.vector.tensor_mul(s[:], xt[l][:], wb[e][:])
            xst.append(s)
        for p in range(PT):
            for l in range(KT):
                nc.tensor.matmul(accs[p][:PW[p], :],
                                 ewt[l][:, p * 128:p * 128 + PW[p]],
                                 xst[l][:],
                                 start=(e == 0 and l == 0), stop=False)

    # ---- bias via matmul, drain, store ----
    for p in range(PT):
        pw = PW[p]
        nc.tensor.matmul(accs[p][:pw, :], ebt[:, p * 128:p * 128 + pw],
                         wT[:], start=False, stop=True)
        ob = ob_pool.tile([128, NROW], F32, tag="ob", name=f"ob{p}")
        nc.scalar.copy(ob[:pw, :], accs[p][:pw, :])
        for b in range(BPC):
            nc.sync.dma_start(out_d[b, p * 128:p * 128 + pw, :],
                              ob[:pw, b * V:(b + 1) * V])


def build_nc():
    from contextlib import ExitStack

    # Bacc (not raw Bass): its compile() legalizes sync waits — walrus
    # codegen allows at most 1 wait per instruction on TRN2.
    nc = bacc.Bacc("TRN2", target_bir_lowering=False, debug=False)
    x = nc.dram_tensor("x", [BPC, L, V], F32, kind="ExternalInput").ap()
    gw = nc.dram_tensor("gate_w", [L, E], F32, kind="ExternalInput").ap()
    gb = nc.dram_tensor("gate_b", [1, E], F32, kind="ExternalInput").ap()
    ew = nc.dram_tensor("expert_w", [E, L, P], F32, kind="ExternalInput").ap()
    eb = nc.dram_tensor("expert_b", [E, P], F32, kind="ExternalInput").ap()
    out = nc.dram_tensor("out", [BPC, P, V], F32, kind="ExternalOutput").ap()
    with tile.TileContext(nc) as tc, ExitStack() as ctx:
        _build(ctx, tc, x, gw, gb, ew, eb, out)
    nc.compile()
    return nc


_NC_CACHE = None


def _get_nc():
    global _NC_CACHE
    if _NC_CACHE is None:
        _NC_CACHE = build_nc()
    return _NC_CACHE


def make_in_maps(x_enc, gate_w, gate_b, expert_w, expert_b):
    x_enc = np.ascontiguousarray(np.asarray(x_enc, dtype=np.float32))
    gate_w = np.ascontiguousarray(np.asarray(gate_w, dtype=np.float32))
    gate_b = np.ascontiguousarray(np.asarray(gate_b, dtype=np.float32)).reshape(1, E)
    expert_w = np.ascontiguousarray(np.asarray(expert_w, dtype=np.float32))
    expert_b = np.ascontiguousarray(np.asarray(expert_b, dtype=np.float32))
    in_maps = []
    for c in range(N_CORES):
        in_maps.append({
            "x": np.ascontiguousarray(x_enc[c * BPC:(c + 1) * BPC]),
            "gate_w": gate_w,
            "gate_b": gate_b,
            "expert_w": expert_w,
            "expert_b": expert_b,
        })
    return in_maps


def kernel(x_enc, gate_w, gate_b, expert_w, expert_b, _trace=False, _trace_kwargs=None):
    nc = _get_nc()
    in_maps = make_in_maps(x_enc, gate_w, gate_b, expert_w, expert_b)
    res = run_bass_kernel_spmd(
        nc, in_maps, list(range(N_CORES)), trace=_trace,
        **(_trace_kwargs or {}),
    )
    out = np.concatenate([np.asarray(res.results[c]["out"]) for c in range(N_CORES)],
                         axis=0)
    if _trace:
        kernel.last_results = res
    return out
